# revision 33
# baseline (speedup 1.0000x reference)
"""Bass/Trainium2 kernel for nn_BayesianResNet_71408126263673.

Grouped per-sample conv: for each of 32 samples i,
  out[i] = conv2d(x[i] [128,32,32], W[i] [128oc,128c,3,3], pad=1, stride=1) + bias[i]

Sharding: b_i (32 samples) split across 8 NeuronCores, 4 samples per core.
Pure data parallel, no collectives.

Per-core kernel: each sample's conv is computed as 9 accumulating matmuls
(one per 3x3 tap) into PSUM:
  out[oc, pix] = sum_{kh,kw} W[:, :, kh, kw].T @ xpad[:, shifted pix]
with K=c=128 (partition/contraction), M=oc=128, N<=512 pixels per PSUM bank.
The input image is zero-padded to 34x34 on the HOST so DMA loads are fully
contiguous. Weights are pre-transposed on the host to [c, kh*kw, oc] so each
tap is a ready-to-use lhsT (stationary operand) tile.

Timeline engineering (iterated against perfetto/NTFF traces; measured
physics each design choice rests on is noted):
- Per-sample SBUF layout is [taps 0-2 | img rows 0-17 | taps 3-8 |
  img rows 16-33] (rows 16/17 duplicated) so a sample streams in as
  contiguous chunks and Tile's address-range dependency tracking releases
  each matmul as soon as ITS tap/rows have landed. First real matmul
  ~10.3us instead of ~11.4us.
- Each HWDGE queue moves ~190 B/ns (~135 each when both run); the SP queue
  reaches line rate ~0.8us after its first issue while the ACT queue ramps
  for ~2.5us; a transfer's completion SEMAPHORE lags its last byte by
  0.5-2us (HBM write-receipt round trip, worst under concurrent load).
- Matmul blocks run sample-interleaved (s0b0, s1b0, s0b1, s1b1, s2b0,
  ...): after the first block the stream consumes data prefetched on the
  OTHER queue, which buys every later chunk >=1.5us of slack against the
  receipt straggle. Mid-stream stalls are doubly toxic: the PE's HAM
  clock-gate re-throttles to 1.2GHz after an idle gap and takes ~3.4us of
  continuous activity to recover (a 2.4us stall measured ~5.4us of loss).
- PE warmup (dependency-free matmuls on an uninitialized raw SBUF tensor,
  so the first LDWEIGHTS waits on nothing) runs from the engine barrier
  until the first data lands with no idle gap, so the 1.2->2.4GHz
  un-throttle usually lands before or just after the real stream starts.
- Outputs are written fp16 (host upcasts; +~1e-4 rel err): halves store
  bytes. The last sample is split 16/8/8 rows so the final ACT is small,
  ACTs into a raw (non-pool) SBUF tensor, and its two store halves are
  emitted AFTER the TileContext exit barrier with a completion sem nobody
  waits on: they drain during the ~7us walrus epilogue (which zeroes all
  256 semaphores engine-by-engine and cannot be shrunk or skipped) and
  retire ~5us before the NEFF's final barrier.
Framework floor (immovable from kernel code): ~0.75us of const memsets +
engine barrier before the first DMA issue, and the ~7.9us epilogue+final
barrier. Compute floor: 72 N=512-equivalent fp16 matmuls = 15.5us warm.
"""

import numpy as np

import concourse.bacc as bacc
import concourse.tile as tile
from concourse import mybir
from concourse.bass_utils import run_bass_kernel_spmd

N_CORES = 8
B_I, B_J, C, H, W = 32, 1, 128, 32, 32
OC, KH, KW = 128, 3, 3
S = B_I // N_CORES            # samples per core
HP, WP = H + 2, W + 2         # padded image
NTAP = KH * KW                # 9

MM_DT = mybir.dt.float16
MM_NP = np.float16
OUT_DT = mybir.dt.float16
X_DT = W_DT = MM_DT  # test.py prints these

# Per-sample column layout (partition dim = C):
#   [taps 0-2 | rows 0-17 | taps 3-8 | rows 16-33]
NT_A = 3                      # taps in the first segment
ROWS_A = 18                   # rows 0..17  (covers block-0 reach)
ROWS_B = 18                   # rows 16..33 (covers block-1 reach)
SEG0 = 0
SEG1 = SEG0 + NT_A * OC       # 384:  rows 0-17 start
SEG2 = SEG1 + ROWS_A * WP     # 996:  taps 3-8 start
SEG3 = SEG2 + (NTAP - NT_A) * OC  # 1764: rows 16-33 start
NCOL = SEG3 + ROWS_B * WP     # 2376 columns total

# Sample-0 DMA chunk boundaries (sequential on the SP queue). The first
# chunk carries taps 0-4 + rows 0-17 so matmuls 0-4 of block 0 are released
# ~0.4us earlier; taps 5-8 follow with ~0.8us of margin before matmul 5.
CH1 = SEG2 + 2 * OC           # taps 0-2 + rows 0-17 + taps 3-4
CH2 = SEG3                    # taps 5-8

# Row-block split per sample: 16+16, except the last sample 16+8+8 so the
# final ACT+store (the serial tail after the last matmul) is half-sized.
BLOCKS = [(0, 16), (16, 16)]
BLOCKS_LAST = [(0, 16), (16, 8), (24, 8)]

N_WARMUP = 29  # ~3.1us of N=128 matmuls; bridges engine start -> first data
               # with no PE idle gap (an idle gap restarts the ~3.4us HAM
               # activity window and the stream re-throttles to 1.2GHz)

# test.py hooks
TRACE = False
TRACE_KW = {}
LAST_RESULTS = None

_NC_CACHE = None


def _build_nc():
    f32 = mybir.dt.float32
    nc = bacc.Bacc()
    xw_d = nc.declare_dram_parameter("xw", [S, C, NCOL], MM_DT, isOutput=False)
    b_d = nc.declare_dram_parameter("b", [OC, S], f32, isOutput=False)
    o_d = nc.declare_dram_parameter("o", [S, OC, H, W], OUT_DT, isOutput=True)

    # Raw (non-pool) SBUF tensor for the last sample's output: its AP is
    # concrete, so the deferred stores below can be emitted after the
    # TileContext exit (pool-tile APs are symbolic and die with the context).
    out_late = nc.alloc_sbuf_tensor("out_late", [OC, H, W], OUT_DT)
    # Raw warmup operand, deliberately uninitialized: the PE's first
    # LDWEIGHTS then has no wait at all and warmup begins right at the
    # engine barrier, pulling the HAM 1.2->2.4GHz un-throttle (a free-
    # running ~3.4us activity window) earlier. PSUM garbage is never read.
    wu_x = nc.alloc_sbuf_tensor("warmup_x", [C, OC], MM_DT)

    with tile.TileContext(nc, pool_alloc_mode="queue") as tc:
        with (
            tc.tile_pool(name="ins", bufs=1) as ins_pool,
            tc.tile_pool(name="outs", bufs=1) as outs_pool,
            tc.tile_pool(name="psum", bufs=8, space="PSUM") as psum_pool,
        ):
            wu_ps = psum_pool.tile([C, OC], f32, name="wu_ps", tag="ps")
            for _ in range(N_WARMUP):
                nc.tensor.matmul(wu_ps[:], wu_x[:], wu_x[:], start=True, stop=True)

            xw_ts = [
                ins_pool.tile([C, NCOL], MM_DT, tag=f"xw{s}", name=f"xw{s}")
                for s in range(S)
            ]
            bias_t = ins_pool.tile([OC, S], f32, tag="bias")

            def tap_view(s, t):
                if t < NT_A:
                    return xw_ts[s][:, t * OC : (t + 1) * OC]
                return xw_ts[s][:, SEG2 + (t - NT_A) * OC : SEG2 + (t - NT_A + 1) * OC]

            # image views: rows 0-17 and rows 16-33 (as local rows 0-17)
            xva = [
                t[:, SEG1:SEG2].rearrange("p (h w) -> p h w", w=WP) for t in xw_ts
            ]
            xvb = [
                t[:, SEG3:].rearrange("p (h w) -> p h w", w=WP) for t in xw_ts
            ]

            # Input streaming. Measured physics: each HWDGE queue moves
            # ~190 B/ns (both active: ~135 each), a ring serves its
            # dma_starts strictly in issue order, and a transfer's completion
            # SEMAPHORE lags its last byte by 0.5-2us (HBM write-receipt
            # round trip, worst under load). Mid-stream stalls additionally
            # re-throttle the PE clock for ~3.4us, so every chunk is
            # scheduled with >=0.7us of sem-side margin:
            #   SP queue:  s0 chunks -> s1 rows (+ stores later)
            #   ACT queue: bias -> s1 tap chunks -> s2 -> s3
            # Matmul blocks run sample-interleaved (s0b0, s1b0, s0b1, s1b1,
            # s2b0, ...), so after block 1 the stream consumes data
            # prefetched on the OTHER queue — every chunk past the first two
            # gets >=1.5us of slack against the receipt straggle.
            nc.sync.dma_start(xw_ts[0][:, :CH1], xw_d[0][:, :CH1])
            nc.scalar.dma_start(bias_t[:], b_d[:])  # tiny; warms the ACT queue
            nc.sync.dma_start(xw_ts[0][:, CH1:CH2], xw_d[0][:, CH1:CH2])
            nc.scalar.dma_start(xw_ts[1][:, :CH1], xw_d[1][:, :CH1])
            nc.sync.dma_start(xw_ts[0][:, CH2:], xw_d[0][:, CH2:])
            nc.scalar.dma_start(xw_ts[1][:, CH1:CH2], xw_d[1][:, CH1:CH2])
            nc.sync.dma_start(xw_ts[1][:, CH2:], xw_d[1][:, CH2:])
            nc.scalar.dma_start(xw_ts[2][:], xw_d[2])
            nc.scalar.dma_start(xw_ts[3][:], xw_d[3])

            def conv_block(s, row0, nrows, ps_name):
                """One accumulation group: output rows [row0, row0+nrows)."""
                ps = psum_pool.tile([OC, nrows, W], f32, name=ps_name, tag="ps")
                xv, base = (xva[s], 0) if row0 + nrows + 2 <= ROWS_A else (xvb[s], 16)
                for t in range(NTAP):
                    kh, kw = divmod(t, KW)
                    r0 = row0 - base + kh
                    rhs = xv[:, r0 : r0 + nrows, kw : kw + W]
                    nc.tensor.matmul(
                        ps[:], tap_view(s, t), rhs,
                        start=(t == 0), stop=(t == NTAP - 1),
                    )
                return ps

            out_ts = {
                s: outs_pool.tile([OC, H, W], OUT_DT, tag=f"out{s}", name=f"out{s}")
                for s in range(S - 1)
            }
            # Sample-interleaved block order (see DMA comment above).
            order = [(0, 0), (1, 0), (0, 1), (1, 1), (2, 0), (2, 1)]
            order += [(S - 1, bi) for bi in range(len(BLOCKS_LAST))]
            for s, bi in order:
                blocks = BLOCKS_LAST if s == S - 1 else BLOCKS
                row0, nrows = blocks[bi]
                ps = conv_block(s, row0, nrows, f"ps{s}_{bi}")
                late = s == S - 1
                if late:
                    # The last sample ACTs into the raw tensor; its stores
                    # are deferred past the tile-exit barrier so nothing
                    # waits on their completion receipts — the ~7us
                    # framework epilogue (which zeroes all 256 sems
                    # engine-by-engine) overlaps their drain instead.
                    src = out_late[:, row0 : row0 + nrows, :]
                else:
                    src = out_ts[s][:, row0 : row0 + nrows, :]
                nc.scalar.activation(
                    src,
                    ps[:],
                    mybir.ActivationFunctionType.Identity,
                    bias=bias_t[:, s : s + 1],
                )
                dst = o_d[s][:, row0 : row0 + nrows, :]
                if late:
                    pass  # stored post-exit as two merged halves (below)
                else:
                    # Early stores ride the SP queue (idle after s0's
                    # chunks; ring order keeps them behind the inputs).
                    nc.sync.dma_start(dst, src)
    # Past TileContext exit: every ACT has retired (tile-exit drain+barrier),
    # so these reads are ordered; their DMAs drain during the epilogue, long
    # before the NEFF's final barrier retires. Codegen requires sync info on
    # DGE transfers, so each gets a completion sem that nothing waits on.
    # One merged half per engine keeps the post-exit issue cost to ~0.6us.
    for i, (rows, eng) in enumerate([(slice(0, 16), nc.sync),
                                     (slice(16, 32), nc.scalar)]):
        sem = nc.alloc_semaphore(f"late_store{i}")
        eng.dma_start(o_d[S - 1][:, rows, :], out_late[:, rows, :]).then_inc(
            sem, 16
        )
    nc.compile()
    return nc


def _get_nc():
    global _NC_CACHE
    if _NC_CACHE is None:
        _NC_CACHE = _build_nc()
    return _NC_CACHE


def kernel(x: np.ndarray, weight: np.ndarray, bias: np.ndarray) -> np.ndarray:
    global LAST_RESULTS
    assert x.shape == (B_I, B_J, C, H, W)
    assert weight.shape == (B_I, OC, C, KH, KW)
    assert bias.shape == (B_I, B_J, OC)

    x = np.asarray(x, dtype=np.float32)
    weight = np.asarray(weight, dtype=np.float32)
    bias = np.asarray(bias, dtype=np.float32)

    # Host-side layout prep (part of sharding): zero-pad images, transpose
    # weights so each 3x3 tap is a contiguous [c, oc] stationary tile.
    # Layout per sample: [taps 0-2 | rows 0-17 | taps 3-8 | rows 16-33].
    wt = weight.transpose(0, 2, 3, 4, 1).reshape(B_I, C, NTAP * OC).astype(MM_NP)
    xpad = np.zeros((B_I, C, HP, WP), dtype=MM_NP)
    xpad[:, :, 1 : 1 + H, 1 : 1 + W] = x[:, 0].astype(MM_NP)

    xw = np.empty((B_I, C, NCOL), dtype=MM_NP)
    xw[:, :, SEG0:SEG1] = wt[:, :, : NT_A * OC]
    xw[:, :, SEG1:SEG2] = xpad[:, :, :ROWS_A].reshape(B_I, C, ROWS_A * WP)
    xw[:, :, SEG2:SEG3] = wt[:, :, NT_A * OC :]
    xw[:, :, SEG3:] = xpad[:, :, HP - ROWS_B :].reshape(B_I, C, ROWS_B * WP)
    bt = bias[:, 0, :]  # [b_i, oc]

    in_maps = []
    for core in range(N_CORES):
        sl = slice(core * S, (core + 1) * S)
        in_maps.append(
            {
                "xw": np.ascontiguousarray(xw[sl]),
                "b": np.ascontiguousarray(bt[sl].T),  # [OC, S]
            }
        )

    nc = _get_nc()
    try:
        res = run_bass_kernel_spmd(
            nc, in_maps, core_ids=list(range(N_CORES)), trace=TRACE, **TRACE_KW
        )
    except Exception:
        # Transient NRT/device errors usually clear on retry; idempotent.
        import time

        time.sleep(10)
        res = run_bass_kernel_spmd(
            nc, in_maps, core_ids=list(range(N_CORES)), trace=TRACE, **TRACE_KW
        )
    LAST_RESULTS = res

    out = np.concatenate(
        [res.results[c]["o"].astype(np.float32) for c in range(N_CORES)], axis=0
    )
    return out.reshape(B_I, B_J, OC, H, W)


# revision 36
# speedup vs baseline: 1.2132x; 1.2132x over previous
"""Bass/Trainium2 kernel for nn_BayesianResNet_71408126263673.

Grouped per-sample conv: for each of 32 samples i,
  out[i] = conv2d(x[i] [128,32,32], W[i] [128oc,128c,3,3], pad=1, stride=1) + bias[i]

Sharding: b_i (32 samples) split across 8 NeuronCores, 4 samples per core.
Pure data parallel, no collectives.

Per-core kernel: each sample's conv is computed as 9 accumulating matmuls
(one per 3x3 tap) into PSUM:
  out[oc, pix] = sum_{kh,kw} W[:, :, kh, kw].T @ xpad[:, shifted pix]
with K=c=128 (partition/contraction), M=oc=128, N<=512 pixels per PSUM bank.
The input image is zero-padded to 34x34 on the HOST so DMA loads are fully
contiguous. Weights are pre-transposed on the host to [c, kh*kw, oc] so each
tap is a ready-to-use lhsT (stationary operand) tile.

Timeline engineering (iterated against perfetto/NTFF traces; measured
physics each design choice rests on is noted):
- Per-sample SBUF layout is [taps 0-2 | img rows 0-17 | taps 3-8 |
  img rows 16-33] (rows 16/17 duplicated) so a sample streams in as
  contiguous chunks and Tile's address-range dependency tracking releases
  each matmul as soon as ITS tap/rows have landed. First real matmul
  ~10.3us instead of ~11.4us.
- Each HWDGE queue moves ~190 B/ns (~135 each when both run); the SP queue
  reaches line rate ~0.8us after its first issue while the ACT queue ramps
  for ~2.5us; a transfer's completion SEMAPHORE lags its last byte by
  0.5-2us (HBM write-receipt round trip, worst under concurrent load).
- Matmul blocks run sample-interleaved (s0b0, s1b0, s0b1, s1b1, s2b0,
  ...): after the first block the stream consumes data prefetched on the
  OTHER queue, which buys every later chunk >=1.5us of slack against the
  receipt straggle. Mid-stream stalls are doubly toxic: the PE's HAM
  clock-gate re-throttles to 1.2GHz after an idle gap and takes ~3.4us of
  continuous activity to recover (a 2.4us stall measured ~5.4us of loss).
- PE warmup (dependency-free matmuls on an uninitialized raw SBUF tensor,
  so the first LDWEIGHTS waits on nothing) runs from the engine barrier
  until the first data lands with no idle gap, so the 1.2->2.4GHz
  un-throttle usually lands before or just after the real stream starts.
- Outputs are written fp16 (host upcasts; +~1e-4 rel err): halves store
  bytes. The last sample is split 16/8/8 rows so the final ACT is small,
  ACTs into a raw (non-pool) SBUF tensor, and its two store halves are
  emitted AFTER the TileContext exit barrier with a completion sem nobody
  waits on: they drain during the ~7us walrus epilogue (which zeroes all
  256 semaphores engine-by-engine and cannot be shrunk or skipped) and
  retire ~5us before the NEFF's final barrier.
Framework floor (immovable from kernel code): ~0.75us of const memsets +
engine barrier before the first DMA issue, and the ~7.9us epilogue+final
barrier. Compute floor: 72 N=512-equivalent fp16 matmuls = 15.5us warm.
"""

import numpy as np

import concourse.bacc as bacc
import concourse.tile as tile
from concourse import mybir
from concourse.bass_utils import run_bass_kernel_spmd

N_CORES = 8
B_I, B_J, C, H, W = 32, 1, 128, 32, 32
OC, KH, KW = 128, 3, 3
S = B_I // N_CORES            # samples per core
HP, WP = H + 2, W + 2         # padded image
NTAP = KH * KW                # 9

MM_DT = mybir.dt.float16
MM_NP = np.float16
OUT_DT = mybir.dt.float16
X_DT = W_DT = MM_DT  # test.py prints these

# Per-sample column layout (partition dim = C):
#   [taps 0-2 | rows 0-17 | taps 3-8 | rows 16-33]
NT_A = 3                      # taps in the first segment
ROWS_A = 18                   # rows 0..17  (covers block-0 reach)
ROWS_B = 18                   # rows 16..33 (covers block-1 reach)
SEG0 = 0
SEG1 = SEG0 + NT_A * OC       # 384:  rows 0-17 start
SEG2 = SEG1 + ROWS_A * WP     # 996:  taps 3-8 start
SEG3 = SEG2 + (NTAP - NT_A) * OC  # 1764: rows 16-33 start
NCOL = SEG3 + ROWS_B * WP     # 2376 columns total

# Sample-0 DMA chunk boundaries (sequential on the SP queue). The first
# chunk carries taps 0-4 + rows 0-17 so matmuls 0-4 of block 0 are released
# ~0.4us earlier; taps 5-8 follow with ~0.8us of margin before matmul 5.
CH1 = SEG2 + 2 * OC           # taps 0-2 + rows 0-17 + taps 3-4
CH2 = SEG3                    # taps 5-8

# Row-block split per sample: 16+16, except the last sample 16+8+8 so the
# final ACT+store (the serial tail after the last matmul) is half-sized.
BLOCKS = [(0, 16), (16, 16)]
BLOCKS_LAST = [(0, 16), (16, 8), (24, 8)]

N_WARMUP = 29  # ~3.1us of N=128 matmuls; bridges engine start -> first data
               # with no PE idle gap (an idle gap restarts the ~3.4us HAM
               # activity window and the stream re-throttles to 1.2GHz)

# test.py hooks
TRACE = False
TRACE_KW = {}
LAST_RESULTS = None

_NC_CACHE = None


def _build_nc():
    f32 = mybir.dt.float32
    nc = bacc.Bacc()
    xw_d = nc.declare_dram_parameter("xw", [S, C, NCOL], MM_DT, isOutput=False)
    b_d = nc.declare_dram_parameter("b", [OC, S], f32, isOutput=False)
    o_d = nc.declare_dram_parameter("o", [S, OC, H, W], OUT_DT, isOutput=True)

    # Raw (non-pool) SBUF tensor for the last sample's output: its AP is
    # concrete, so the deferred stores below can be emitted after the
    # TileContext exit (pool-tile APs are symbolic and die with the context).
    out_late = nc.alloc_sbuf_tensor("out_late", [OC, H, W], OUT_DT)
    # Raw warmup operand, deliberately uninitialized: the PE's first
    # LDWEIGHTS then has no wait at all and warmup begins right at the
    # engine barrier, pulling the HAM 1.2->2.4GHz un-throttle (a free-
    # running ~3.4us activity window) earlier. PSUM garbage is never read.
    wu_x = nc.alloc_sbuf_tensor("warmup_x", [C, OC], MM_DT)

    with tile.TileContext(nc, pool_alloc_mode="queue") as tc:
        with (
            tc.tile_pool(name="ins", bufs=1) as ins_pool,
            tc.tile_pool(name="outs", bufs=1) as outs_pool,
            tc.tile_pool(name="psum", bufs=8, space="PSUM") as psum_pool,
        ):
            wu_ps = psum_pool.tile([C, OC], f32, name="wu_ps", tag="ps")
            for _ in range(N_WARMUP):
                nc.tensor.matmul(wu_ps[:], wu_x[:], wu_x[:], start=True, stop=True)

            xw_ts = [
                ins_pool.tile([C, NCOL], MM_DT, tag=f"xw{s}", name=f"xw{s}")
                for s in range(S)
            ]
            bias_t = ins_pool.tile([OC, S], f32, tag="bias")

            def tap_view(s, t):
                if t < NT_A:
                    return xw_ts[s][:, t * OC : (t + 1) * OC]
                return xw_ts[s][:, SEG2 + (t - NT_A) * OC : SEG2 + (t - NT_A + 1) * OC]

            # image views: rows 0-17 and rows 16-33 (as local rows 0-17)
            xva = [
                t[:, SEG1:SEG2].rearrange("p (h w) -> p h w", w=WP) for t in xw_ts
            ]
            xvb = [
                t[:, SEG3:].rearrange("p (h w) -> p h w", w=WP) for t in xw_ts
            ]

            # Input streaming. Measured physics: each HWDGE queue moves
            # ~190 B/ns (both active: ~135 each), a ring serves its
            # dma_starts strictly in issue order, and a transfer's completion
            # SEMAPHORE lags its last byte by 0.5-2us (HBM write-receipt
            # round trip, worst under load). Mid-stream stalls additionally
            # re-throttle the PE clock for ~3.4us, so every chunk is
            # scheduled with >=0.7us of sem-side margin:
            #   SP queue:  s0 chunks -> s1 rows (+ stores later)
            #   ACT queue: bias -> s1 tap chunks -> s2 -> s3
            # Matmul blocks run sample-interleaved (s0b0, s1b0, s0b1, s1b1,
            # s2b0, ...), so after block 1 the stream consumes data
            # prefetched on the OTHER queue — every chunk past the first two
            # gets >=1.5us of slack against the receipt straggle.
            nc.sync.dma_start(xw_ts[0][:, :CH1], xw_d[0][:, :CH1])
            nc.scalar.dma_start(bias_t[:], b_d[:])  # tiny; warms the ACT queue
            nc.sync.dma_start(xw_ts[0][:, CH1:CH2], xw_d[0][:, CH1:CH2])
            nc.scalar.dma_start(xw_ts[1][:, :CH1], xw_d[1][:, :CH1])
            nc.sync.dma_start(xw_ts[0][:, CH2:], xw_d[0][:, CH2:])
            nc.scalar.dma_start(xw_ts[1][:, CH1:CH2], xw_d[1][:, CH1:CH2])
            nc.sync.dma_start(xw_ts[1][:, CH2:], xw_d[1][:, CH2:])
            nc.scalar.dma_start(xw_ts[2][:], xw_d[2])
            nc.scalar.dma_start(xw_ts[3][:], xw_d[3])

            def conv_block(s, row0, nrows, ps_name):
                """One accumulation group: output rows [row0, row0+nrows)."""
                ps = psum_pool.tile([OC, nrows, W], f32, name=ps_name, tag="ps")
                xv, base = (xva[s], 0) if row0 + nrows + 2 <= ROWS_A else (xvb[s], 16)
                for t in range(NTAP):
                    kh, kw = divmod(t, KW)
                    r0 = row0 - base + kh
                    rhs = xv[:, r0 : r0 + nrows, kw : kw + W]
                    nc.tensor.matmul(
                        ps[:], tap_view(s, t), rhs,
                        start=(t == 0), stop=(t == NTAP - 1),
                    )
                return ps

            out_ts = {
                s: outs_pool.tile([OC, H, W], OUT_DT, tag=f"out{s}", name=f"out{s}")
                for s in range(S - 1)
            }
            # Sample-interleaved block order (see DMA comment above).
            order = [(0, 0), (1, 0), (0, 1), (1, 1), (2, 0), (2, 1)]
            order += [(S - 1, bi) for bi in range(len(BLOCKS_LAST))]
            for s, bi in order:
                blocks = BLOCKS_LAST if s == S - 1 else BLOCKS
                row0, nrows = blocks[bi]
                ps = conv_block(s, row0, nrows, f"ps{s}_{bi}")
                late = s == S - 1
                if late:
                    # The last sample ACTs into the raw tensor; its stores
                    # are deferred past the tile-exit barrier so nothing
                    # waits on their completion receipts — the ~7us
                    # framework epilogue (which zeroes all 256 sems
                    # engine-by-engine) overlaps their drain instead.
                    src = out_late[:, row0 : row0 + nrows, :]
                else:
                    src = out_ts[s][:, row0 : row0 + nrows, :]
                nc.scalar.activation(
                    src,
                    ps[:],
                    mybir.ActivationFunctionType.Identity,
                    bias=bias_t[:, s : s + 1],
                )
                dst = o_d[s][:, row0 : row0 + nrows, :]
                if late:
                    pass  # stored post-exit as two merged halves (below)
                else:
                    # Early stores ride the SP queue (idle after s0's
                    # chunks; ring order keeps them behind the inputs).
                    nc.sync.dma_start(dst, src)
    # Past TileContext exit: every ACT has retired (tile-exit drain+barrier),
    # so this read is ordered; the DMA drains during the epilogue, long
    # before the NEFF's final barrier retires. Codegen requires sync info on
    # DGE transfers, so it gets a completion sem that nothing waits on.
    # A single merged store on SP keeps ACT's post-exit path empty, so only
    # one engine pays the ~1us issue+drain before the epilogue barrier.
    sem = nc.alloc_semaphore("late_store")
    nc.sync.dma_start(o_d[S - 1], out_late[:]).then_inc(sem, 16)
    nc.compile()
    return nc


def _get_nc():
    global _NC_CACHE
    if _NC_CACHE is None:
        _NC_CACHE = _build_nc()
    return _NC_CACHE


def kernel(x: np.ndarray, weight: np.ndarray, bias: np.ndarray) -> np.ndarray:
    global LAST_RESULTS
    assert x.shape == (B_I, B_J, C, H, W)
    assert weight.shape == (B_I, OC, C, KH, KW)
    assert bias.shape == (B_I, B_J, OC)

    x = np.asarray(x, dtype=np.float32)
    weight = np.asarray(weight, dtype=np.float32)
    bias = np.asarray(bias, dtype=np.float32)

    # Host-side layout prep (part of sharding): zero-pad images, transpose
    # weights so each 3x3 tap is a contiguous [c, oc] stationary tile.
    # Layout per sample: [taps 0-2 | rows 0-17 | taps 3-8 | rows 16-33].
    wt = weight.transpose(0, 2, 3, 4, 1).reshape(B_I, C, NTAP * OC).astype(MM_NP)
    xpad = np.zeros((B_I, C, HP, WP), dtype=MM_NP)
    xpad[:, :, 1 : 1 + H, 1 : 1 + W] = x[:, 0].astype(MM_NP)

    xw = np.empty((B_I, C, NCOL), dtype=MM_NP)
    xw[:, :, SEG0:SEG1] = wt[:, :, : NT_A * OC]
    xw[:, :, SEG1:SEG2] = xpad[:, :, :ROWS_A].reshape(B_I, C, ROWS_A * WP)
    xw[:, :, SEG2:SEG3] = wt[:, :, NT_A * OC :]
    xw[:, :, SEG3:] = xpad[:, :, HP - ROWS_B :].reshape(B_I, C, ROWS_B * WP)
    bt = bias[:, 0, :]  # [b_i, oc]

    in_maps = []
    for core in range(N_CORES):
        sl = slice(core * S, (core + 1) * S)
        in_maps.append(
            {
                "xw": np.ascontiguousarray(xw[sl]),
                "b": np.ascontiguousarray(bt[sl].T),  # [OC, S]
            }
        )

    nc = _get_nc()
    try:
        res = run_bass_kernel_spmd(
            nc, in_maps, core_ids=list(range(N_CORES)), trace=TRACE, **TRACE_KW
        )
    except Exception:
        # Transient NRT/device errors usually clear on retry; idempotent.
        import time

        time.sleep(10)
        res = run_bass_kernel_spmd(
            nc, in_maps, core_ids=list(range(N_CORES)), trace=TRACE, **TRACE_KW
        )
    LAST_RESULTS = res

    out = np.concatenate(
        [res.results[c]["o"].astype(np.float32) for c in range(N_CORES)], axis=0
    )
    return out.reshape(B_I, B_J, OC, H, W)


# revision 37
# speedup vs baseline: 1.2235x; 1.0085x over previous
"""Bass/Trainium2 kernel for nn_BayesianResNet_71408126263673.

Grouped per-sample conv: for each of 32 samples i,
  out[i] = conv2d(x[i] [128,32,32], W[i] [128oc,128c,3,3], pad=1, stride=1) + bias[i]

Sharding: b_i (32 samples) split across 8 NeuronCores, 4 samples per core.
Pure data parallel, no collectives.

Per-core kernel: each sample's conv is computed as 9 accumulating matmuls
(one per 3x3 tap) into PSUM:
  out[oc, pix] = sum_{kh,kw} W[:, :, kh, kw].T @ xpad[:, shifted pix]
with K=c=128 (partition/contraction), M=oc=128, N<=512 pixels per PSUM bank.
The input image is zero-padded to 34x34 on the HOST so DMA loads are fully
contiguous. Weights are pre-transposed on the host to [c, kh*kw, oc] so each
tap is a ready-to-use lhsT (stationary operand) tile.

Timeline engineering (iterated against perfetto/NTFF traces; measured
physics each design choice rests on is noted):
- Per-sample SBUF layout is [taps 0-2 | img rows 0-17 | taps 3-8 |
  img rows 16-33] (rows 16/17 duplicated) so a sample streams in as
  contiguous chunks and Tile's address-range dependency tracking releases
  each matmul as soon as ITS tap/rows have landed. First real matmul
  ~10.3us instead of ~11.4us.
- Each HWDGE queue moves ~190 B/ns (~135 each when both run); the SP queue
  reaches line rate ~0.8us after its first issue while the ACT queue ramps
  for ~2.5us; a transfer's completion SEMAPHORE lags its last byte by
  0.5-2us (HBM write-receipt round trip, worst under concurrent load).
- Matmul blocks run sample-interleaved (s0b0, s1b0, s0b1, s1b1, s2b0,
  ...): after the first block the stream consumes data prefetched on the
  OTHER queue, which buys every later chunk >=1.5us of slack against the
  receipt straggle. Mid-stream stalls are doubly toxic: the PE's HAM
  clock-gate re-throttles to 1.2GHz after an idle gap and takes ~3.4us of
  continuous activity to recover (a 2.4us stall measured ~5.4us of loss).
- PE warmup (dependency-free matmuls on an uninitialized raw SBUF tensor,
  so the first LDWEIGHTS waits on nothing) runs from the engine barrier
  until the first data lands with no idle gap, so the 1.2->2.4GHz
  un-throttle usually lands before or just after the real stream starts.
- Outputs are written fp16 (host upcasts; +~1e-4 rel err): halves store
  bytes. The last sample is split 16/8/8 rows so the final ACT is small,
  ACTs into a raw (non-pool) SBUF tensor, and its two store halves are
  emitted AFTER the TileContext exit barrier with a completion sem nobody
  waits on: they drain during the ~7us walrus epilogue (which zeroes all
  256 semaphores engine-by-engine and cannot be shrunk or skipped) and
  retire ~5us before the NEFF's final barrier.
Framework floor (immovable from kernel code): ~0.75us of const memsets +
engine barrier before the first DMA issue, and the ~7.9us epilogue+final
barrier. Compute floor: 72 N=512-equivalent fp16 matmuls = 15.5us warm.
"""

import numpy as np

import concourse.bacc as bacc
import concourse.tile as tile
from concourse import mybir
from concourse.bass_utils import run_bass_kernel_spmd

N_CORES = 8
B_I, B_J, C, H, W = 32, 1, 128, 32, 32
OC, KH, KW = 128, 3, 3
S = B_I // N_CORES            # samples per core
HP, WP = H + 2, W + 2         # padded image
NTAP = KH * KW                # 9

MM_DT = mybir.dt.float16
MM_NP = np.float16
OUT_DT = mybir.dt.float16
X_DT = W_DT = MM_DT  # test.py prints these

# Per-sample column layout (partition dim = C):
#   [taps 0-2 | rows 0-17 | taps 3-8 | rows 16-33]
NT_A = 3                      # taps in the first segment
ROWS_A = 18                   # rows 0..17  (covers block-0 reach)
ROWS_B = 18                   # rows 16..33 (covers block-1 reach)
SEG0 = 0
SEG1 = SEG0 + NT_A * OC       # 384:  rows 0-17 start
SEG2 = SEG1 + ROWS_A * WP     # 996:  taps 3-8 start
SEG3 = SEG2 + (NTAP - NT_A) * OC  # 1764: rows 16-33 start
NCOL = SEG3 + ROWS_B * WP     # 2376 columns total

# Sample-0 DMA chunk boundaries (sequential on the SP queue). The first
# chunk carries taps 0-4 + rows 0-17 so matmuls 0-4 of block 0 are released
# ~0.4us earlier; taps 5-8 follow with ~0.8us of margin before matmul 5.
CH1 = SEG2 + 2 * OC           # taps 0-2 + rows 0-17 + taps 3-4
CH2 = SEG3                    # taps 5-8

# Row-block split per sample: 16+16, except the last sample 16+8+8 so the
# final ACT+store (the serial tail after the last matmul) is half-sized.
BLOCKS = [(0, 16), (16, 16)]
BLOCKS_LAST = [(0, 16), (16, 8), (24, 4), (28, 4)]

N_WARMUP = 29  # ~3.1us of N=128 matmuls; bridges engine start -> first data
               # with no PE idle gap (an idle gap restarts the ~3.4us HAM
               # activity window and the stream re-throttles to 1.2GHz)

# test.py hooks
TRACE = False
TRACE_KW = {}
LAST_RESULTS = None

_NC_CACHE = None


def _build_nc():
    f32 = mybir.dt.float32
    nc = bacc.Bacc()
    xw_d = nc.declare_dram_parameter("xw", [S, C, NCOL], MM_DT, isOutput=False)
    b_d = nc.declare_dram_parameter("b", [OC, S], f32, isOutput=False)
    o_d = nc.declare_dram_parameter("o", [S, OC, H, W], OUT_DT, isOutput=True)

    # Raw (non-pool) SBUF tensor for the last sample's output: its AP is
    # concrete, so the deferred stores below can be emitted after the
    # TileContext exit (pool-tile APs are symbolic and die with the context).
    out_late = nc.alloc_sbuf_tensor("out_late", [OC, H, W], OUT_DT)
    # Raw warmup operand, deliberately uninitialized: the PE's first
    # LDWEIGHTS then has no wait at all and warmup begins right at the
    # engine barrier, pulling the HAM 1.2->2.4GHz un-throttle (a free-
    # running ~3.4us activity window) earlier. PSUM garbage is never read.
    wu_x = nc.alloc_sbuf_tensor("warmup_x", [C, OC], MM_DT)

    with tile.TileContext(nc, pool_alloc_mode="queue") as tc:
        with (
            tc.tile_pool(name="ins", bufs=1) as ins_pool,
            tc.tile_pool(name="outs", bufs=1) as outs_pool,
            tc.tile_pool(name="psum", bufs=8, space="PSUM") as psum_pool,
        ):
            wu_ps = psum_pool.tile([C, OC], f32, name="wu_ps", tag="ps")
            for _ in range(N_WARMUP):
                nc.tensor.matmul(wu_ps[:], wu_x[:], wu_x[:], start=True, stop=True)

            xw_ts = [
                ins_pool.tile([C, NCOL], MM_DT, tag=f"xw{s}", name=f"xw{s}")
                for s in range(S)
            ]
            bias_t = ins_pool.tile([OC, S], f32, tag="bias")

            def tap_view(s, t):
                if t < NT_A:
                    return xw_ts[s][:, t * OC : (t + 1) * OC]
                return xw_ts[s][:, SEG2 + (t - NT_A) * OC : SEG2 + (t - NT_A + 1) * OC]

            # image views: rows 0-17 and rows 16-33 (as local rows 0-17)
            xva = [
                t[:, SEG1:SEG2].rearrange("p (h w) -> p h w", w=WP) for t in xw_ts
            ]
            xvb = [
                t[:, SEG3:].rearrange("p (h w) -> p h w", w=WP) for t in xw_ts
            ]

            # Input streaming. Measured physics: each HWDGE queue moves
            # ~190 B/ns (both active: ~135 each), a ring serves its
            # dma_starts strictly in issue order, and a transfer's completion
            # SEMAPHORE lags its last byte by 0.5-2us (HBM write-receipt
            # round trip, worst under load). Mid-stream stalls additionally
            # re-throttle the PE clock for ~3.4us, so every chunk is
            # scheduled with >=0.7us of sem-side margin:
            #   SP queue:  s0 chunks -> s1 rows (+ stores later)
            #   ACT queue: bias -> s1 tap chunks -> s2 -> s3
            # Matmul blocks run sample-interleaved (s0b0, s1b0, s0b1, s1b1,
            # s2b0, ...), so after block 1 the stream consumes data
            # prefetched on the OTHER queue — every chunk past the first two
            # gets >=1.5us of slack against the receipt straggle.
            nc.sync.dma_start(xw_ts[0][:, :CH1], xw_d[0][:, :CH1])
            nc.scalar.dma_start(bias_t[:], b_d[:])  # tiny; warms the ACT queue
            nc.sync.dma_start(xw_ts[0][:, CH1:CH2], xw_d[0][:, CH1:CH2])
            nc.scalar.dma_start(xw_ts[1][:, :CH1], xw_d[1][:, :CH1])
            nc.sync.dma_start(xw_ts[0][:, CH2:], xw_d[0][:, CH2:])
            nc.scalar.dma_start(xw_ts[1][:, CH1:CH2], xw_d[1][:, CH1:CH2])
            nc.sync.dma_start(xw_ts[1][:, CH2:], xw_d[1][:, CH2:])
            nc.scalar.dma_start(xw_ts[2][:], xw_d[2])
            nc.scalar.dma_start(xw_ts[3][:], xw_d[3])

            def conv_block(s, row0, nrows, ps_name):
                """One accumulation group: output rows [row0, row0+nrows)."""
                ps = psum_pool.tile([OC, nrows, W], f32, name=ps_name, tag="ps")
                xv, base = (xva[s], 0) if row0 + nrows + 2 <= ROWS_A else (xvb[s], 16)
                for t in range(NTAP):
                    kh, kw = divmod(t, KW)
                    r0 = row0 - base + kh
                    rhs = xv[:, r0 : r0 + nrows, kw : kw + W]
                    nc.tensor.matmul(
                        ps[:], tap_view(s, t), rhs,
                        start=(t == 0), stop=(t == NTAP - 1),
                    )
                return ps

            out_ts = {
                s: outs_pool.tile([OC, H, W], OUT_DT, tag=f"out{s}", name=f"out{s}")
                for s in range(S - 1)
            }
            # Sample-interleaved block order (see DMA comment above).
            order = [(0, 0), (1, 0), (0, 1), (1, 1), (2, 0), (2, 1)]
            order += [(S - 1, bi) for bi in range(len(BLOCKS_LAST))]
            for s, bi in order:
                blocks = BLOCKS_LAST if s == S - 1 else BLOCKS
                row0, nrows = blocks[bi]
                ps = conv_block(s, row0, nrows, f"ps{s}_{bi}")
                late = s == S - 1
                if late:
                    # The last sample ACTs into the raw tensor; its stores
                    # are deferred past the tile-exit barrier so nothing
                    # waits on their completion receipts — the ~7us
                    # framework epilogue (which zeroes all 256 sems
                    # engine-by-engine) overlaps their drain instead.
                    src = out_late[:, row0 : row0 + nrows, :]
                else:
                    src = out_ts[s][:, row0 : row0 + nrows, :]
                nc.scalar.activation(
                    src,
                    ps[:],
                    mybir.ActivationFunctionType.Identity,
                    bias=bias_t[:, s : s + 1],
                )
                dst = o_d[s][:, row0 : row0 + nrows, :]
                if late:
                    pass  # stored post-exit as two merged halves (below)
                else:
                    # Early stores ride the SP queue (idle after s0's
                    # chunks; ring order keeps them behind the inputs).
                    nc.sync.dma_start(dst, src)
    # Past TileContext exit: every ACT has retired (tile-exit drain+barrier),
    # so this read is ordered; the DMA drains during the epilogue, long
    # before the NEFF's final barrier retires. Codegen requires sync info on
    # DGE transfers, so it gets a completion sem that nothing waits on.
    # A single merged store on SP keeps ACT's post-exit path empty, so only
    # one engine pays the ~1us issue+drain before the epilogue barrier.
    sem = nc.alloc_semaphore("late_store")
    nc.sync.dma_start(o_d[S - 1], out_late[:]).then_inc(sem, 16)
    nc.compile()
    return nc


def _get_nc():
    global _NC_CACHE
    if _NC_CACHE is None:
        _NC_CACHE = _build_nc()
    return _NC_CACHE


def kernel(x: np.ndarray, weight: np.ndarray, bias: np.ndarray) -> np.ndarray:
    global LAST_RESULTS
    assert x.shape == (B_I, B_J, C, H, W)
    assert weight.shape == (B_I, OC, C, KH, KW)
    assert bias.shape == (B_I, B_J, OC)

    x = np.asarray(x, dtype=np.float32)
    weight = np.asarray(weight, dtype=np.float32)
    bias = np.asarray(bias, dtype=np.float32)

    # Host-side layout prep (part of sharding): zero-pad images, transpose
    # weights so each 3x3 tap is a contiguous [c, oc] stationary tile.
    # Layout per sample: [taps 0-2 | rows 0-17 | taps 3-8 | rows 16-33].
    wt = weight.transpose(0, 2, 3, 4, 1).reshape(B_I, C, NTAP * OC).astype(MM_NP)
    xpad = np.zeros((B_I, C, HP, WP), dtype=MM_NP)
    xpad[:, :, 1 : 1 + H, 1 : 1 + W] = x[:, 0].astype(MM_NP)

    xw = np.empty((B_I, C, NCOL), dtype=MM_NP)
    xw[:, :, SEG0:SEG1] = wt[:, :, : NT_A * OC]
    xw[:, :, SEG1:SEG2] = xpad[:, :, :ROWS_A].reshape(B_I, C, ROWS_A * WP)
    xw[:, :, SEG2:SEG3] = wt[:, :, NT_A * OC :]
    xw[:, :, SEG3:] = xpad[:, :, HP - ROWS_B :].reshape(B_I, C, ROWS_B * WP)
    bt = bias[:, 0, :]  # [b_i, oc]

    in_maps = []
    for core in range(N_CORES):
        sl = slice(core * S, (core + 1) * S)
        in_maps.append(
            {
                "xw": np.ascontiguousarray(xw[sl]),
                "b": np.ascontiguousarray(bt[sl].T),  # [OC, S]
            }
        )

    nc = _get_nc()
    try:
        res = run_bass_kernel_spmd(
            nc, in_maps, core_ids=list(range(N_CORES)), trace=TRACE, **TRACE_KW
        )
    except Exception:
        # Transient NRT/device errors usually clear on retry; idempotent.
        import time

        time.sleep(10)
        res = run_bass_kernel_spmd(
            nc, in_maps, core_ids=list(range(N_CORES)), trace=TRACE, **TRACE_KW
        )
    LAST_RESULTS = res

    out = np.concatenate(
        [res.results[c]["o"].astype(np.float32) for c in range(N_CORES)], axis=0
    )
    return out.reshape(B_I, B_J, OC, H, W)
